# revision 35
# baseline (speedup 1.0000x reference)
"""GAT layer (PyG GATConv-style) on 8 Trainium2 NeuronCores.

Strategy:
- Nodes sharded across 8 cores by destination; edges partitioned by destination
  node in a partition-aligned layout: each destination node owns one SBUF
  partition of its block; its incoming edges sit along the free dim, padded to
  a per-block uniform length L (degree-sorted bin packing keeps padding ~1.3%).
- Host expands x[src] per edge slot in fp8-e4m3 (sharding-time data movement)
  with a k-half-interleaved layout [pf][kh][l][p] that doubles as the DoubleRow
  matmul operand layout: per-edge h and a_src each take ONE DoubleRow fp8
  matmul (k=256 in a single pass, 0.5 cyc/row). Folded weights are fp8-e4m3
  scaled x4 to dodge subnormals; the 1/4 is undone exactly via the exp scale
  (attention path) and the 1/s normalization (h path).
- Every node's self-loop sits at edge column 0, so the edge stream itself
  carries each node's own features: a_dst pre-fills the block's a_src PSUM
  bank by a matmul with lhsT = the l=0 slice and a broadcast-tiled Wtd rhs
  (no separate own-feature input, no logits add), and one extra DoubleRow
  matmul drops per-node a_dst into spare agg-bank columns for the padding
  correction. ACT computes e = exp(leaky(z)) straight from PSUM via
  Prelu+Exp (both live in one activation table - no reloads).
- Padding slots are exact zeros; their softmax contribution is subtracted
  analytically (host pad-count x device exp(leaky(a_dst))).
- Weighted aggregation: PSUM-accumulated identity matmuls - DoubleRow fp8
  pairs for the h part, bf16 singles for the e part.
- The log-softmax finalize runs in 7-block groups overlapped under the main
  loop (Pool does the fp32 elementwise, ACT the exp and PSUM->SBUF moves);
  an early flush ships the first 63 blocks' fp16 output while the tail still
  computes.

kernel(**inputs) takes FULL inputs and returns the FULL [N, 64] output.
"""

import numpy as np
import ml_dtypes

import concourse.bass as bass
import concourse.bacc as bacc
import concourse.tile as tile
from concourse import mybir
from concourse.bass_utils import run_bass_kernel_spmd
from concourse.masks import make_identity

# Problem shape (hardcoded per contract)
N, F, E = 100000, 256, 1600000
H, C = 8, 8
HC = H * C  # 64
NEG_SLOPE = 0.2
SC_H = 4.0   # fp8 weight scale for the h projection
SC_A = 4.0   # fp8/bf16 weight scale for the attention projections

P = 128
NCORES = 8
NB = 98                      # blocks per core
NPC = NB * P                 # 12544 node slots per core
NSLOT = NCORES * NPC         # 100352 >= N
TD = HC + H                  # 72: agg layout [h(64) | e(8)]
TD2 = TD + H                 # 80: agg layout [h | e | a_dst(8)]

bf16 = ml_dtypes.bfloat16
f8 = ml_dtypes.float8_e4m3


def _host_prep(x, edge_index, W, att_src, att_dst, bias):
    src_e = np.asarray(edge_index[0], dtype=np.int64)
    dst_e = np.asarray(edge_index[1], dtype=np.int64)
    loop = np.arange(N, dtype=np.int64)
    # self-loops FIRST so each node's own features sit at edge column l=0
    src = np.concatenate([loop, src_e])
    dst = np.concatenate([loop, dst_e])

    deg = np.bincount(dst, minlength=N).astype(np.int64)

    # nodes sorted by degree desc -> global 128-slot blocks dealt round-robin
    # to cores so every core's j-th block has (nearly) equal max degree.
    order = np.argsort(-deg, kind="stable")
    ks = np.arange(NSLOT)
    g = ks // P
    p = ks % P
    c = g % NCORES
    j = g // NCORES
    rows = c * NPC + j * P + p          # device row of global sorted slot k
    row2node = np.full(NSLOT, -1, dtype=np.int64)
    row2node[rows[:N]] = order
    node2row = np.empty(N, dtype=np.int64)
    node2row[order] = rows[:N]

    # per-core-block uniform L schedule (exact max over the 8-block group)
    deg_slot = np.zeros(NSLOT, dtype=np.int64)
    deg_slot[:N] = deg[order]           # degree of global sorted slot k
    degb = deg_slot.reshape(NSLOT // P, P).max(axis=1)   # per global block g
    L_sched = degb.reshape(NB, NCORES).max(axis=1)
    L_sched = np.maximum(L_sched, 1)
    assert L_sched.max() <= 63, "a_src PSUM bank holds at most 63 edge columns"
    off = np.zeros(NB + 1, dtype=np.int64)
    off[1:] = np.cumsum(P * L_sched)
    S = int(off[-1])                    # edge slots per core

    # pad count per device row (for the analytic softmax-denominator fix)
    deg_row = np.zeros(NSLOT, dtype=np.int64)
    deg_row[rows] = deg_slot

    # folded weights
    Wt = np.asarray(W, dtype=np.float64).T            # [256, 64]
    att_s = np.asarray(att_src, np.float64)           # [8, 8]
    att_d = np.asarray(att_dst, np.float64)
    Wts = np.stack([Wt[:, h * C:(h + 1) * C] @ att_s[h] for h in range(H)], axis=1)  # [256, 8]
    Wtd = np.stack([Wt[:, h * C:(h + 1) * C] @ att_d[h] for h in range(H)], axis=1)  # [256, 8]

    def il(a, dtype):  # feature-interleave rows: [256, d] -> [128, 2, d]
        d = a.shape[1]
        return np.ascontiguousarray(
            a.reshape(2, P, d).transpose(1, 0, 2).astype(np.float32).astype(dtype))

    wf8 = il(Wt * SC_H, f8)            # [128, 2, 64] fp8, x4
    was8 = il(Wts * SC_A, f8)          # [128, 2, 8]  fp8, x4
    wtd8 = il(Wtd * SC_A, f8)          # [128, 2, 8]  fp8, x4 (per-node a_dst)
    wdb = il(Wtd * SC_A, bf16)         # [128, 2, 8]  bf16, x4 (a_dst prefill)
    assert np.abs(Wt * SC_H).max() < 400 and np.abs(Wts * SC_A).max() < 400

    # DoubleRow identity: idr[p, i, m] = (m == p) for both k-tiles i
    idr = np.zeros((P, 2, P), dtype=f8)
    idr[np.arange(P), :, np.arange(P)] = 1.0

    # edge -> slot (vectorized); slot storage order (j, l, p)
    eorder = np.argsort(dst, kind="stable")
    dst_s = dst[eorder]
    src_s = src[eorder]
    starts = np.zeros(N + 1, dtype=np.int64)
    starts[1:] = np.cumsum(deg)
    l_rank = np.arange(len(dst_s), dtype=np.int64) - starts[dst_s]
    r = node2row[dst_s]
    ec = r // NPC
    within = r % NPC
    ej = within // P
    ep = within % P
    pos = off[ej] + l_rank * P + ep

    x_f8 = np.asarray(x, np.float32).astype(f8)
    assert np.abs(np.asarray(x, np.float32)).max() < 400  # e4m3 range

    bias_rep = np.tile(np.asarray(bias, np.float32).reshape(1, HC), (P, 1))

    in_maps = []
    for cc in range(NCORES):
        m = ec == cc
        xe = np.zeros((S, F), dtype=f8)               # pad slots stay zero
        xe[pos[m]] = x_f8[src_s[m]]
        # per block: [L, P, F] -> [F, L*P] -> k-half interleave [128, 2, L*P]
        parts = []
        for jj in range(NB):
            Lj = int(L_sched[jj])
            a = xe[off[jj]:off[jj + 1]].reshape(Lj, P, F)      # [l, p, f]
            a = a.transpose(2, 0, 1).reshape(2, P, Lj * P)     # [kh*128f, l*p]
            a = a.transpose(1, 0, 2)                           # [128f, kh, l*p]
            parts.append(np.ascontiguousarray(a).reshape(-1))
        xeT = np.concatenate(parts)
        del xe

        # pad slots per row: [P, NB]
        d = deg_row[cc * NPC:(cc + 1) * NPC].reshape(NB, P)
        npad = (L_sched[:, None] - d).T.astype(np.float32).astype(bf16)

        in_maps.append({
            "xeT": xeT,
            "wf8": wf8,
            "was8": was8,
            "wtd8": wtd8,
            "wdb": wdb,
            "idr": np.ascontiguousarray(idr.reshape(P, 2 * P)),
            "bias_rep": bias_rep,
            "npad": np.ascontiguousarray(npad),
        })
    return in_maps, L_sched, S, row2node


def _build_program(L_sched, S, BX=5, BH=4, B2=4, KPRE=4):
    nc = bacc.Bacc("TRN2", target_bir_lowering=False, debug=False,
                   enable_asserts=False, num_devices=NCORES)
    dt = mybir.dt
    DR = mybir.MatmulPerfMode.DoubleRow

    xeT = nc.dram_tensor("xeT", [S * 2 * P], dt.float8e4, kind="ExternalInput").ap()
    wf8 = nc.dram_tensor("wf8", [P, 2, HC], dt.float8e4, kind="ExternalInput").ap()
    was8 = nc.dram_tensor("was8", [P, 2, H], dt.float8e4, kind="ExternalInput").ap()
    wtd8 = nc.dram_tensor("wtd8", [P, 2, H], dt.float8e4, kind="ExternalInput").ap()
    wdb = nc.dram_tensor("wdb", [P, 2, H], dt.bfloat16, kind="ExternalInput").ap()
    idr = nc.dram_tensor("idr", [P, 2 * P], dt.float8e4, kind="ExternalInput").ap()
    bias_rep = nc.dram_tensor("bias_rep", [P, HC], dt.float32, kind="ExternalInput").ap()
    npad = nc.dram_tensor("npad", [P, NB], dt.bfloat16, kind="ExternalInput").ap()
    out = nc.dram_tensor("out", [P, NB * HC], dt.float16, kind="ExternalOutput").ap()

    AF = mybir.ActivationFunctionType
    OP = mybir.AluOpType
    GP8 = 8   # l-group: 8 x 64 fp32 fills one 2KB PSUM bank
    GB = 14   # finalize group: blocks normalized together, overlapping the loop
    NCUT = 70   # early-flush boundary (5 groups); tail keeps the last 28 blocks
    ISC = 1.0 / SC_A   # exp input scale undoing the x4 attention-weight scale

    with tile.TileContext(nc) as tc:
        with (
            tc.tile_pool(name="const", bufs=1) as constp,
            tc.tile_pool(name="resid", bufs=1) as residp,
            tc.tile_pool(name="p2xpre", bufs=KPRE) as p2xpre,
        ):
            wf8_t = constp.tile([P, 2, HC], dt.float8e4)
            nc.sync.dma_start(wf8_t[:], wf8[:])
            was8_t = constp.tile([P, 2, H], dt.float8e4)
            nc.sync.dma_start(was8_t[:], was8[:])
            wtd8_t = constp.tile([P, 2, H], dt.float8e4)
            nc.sync.dma_start(wtd8_t[:], wtd8[:])
            wdb_t = constp.tile([P, 2, H], dt.bfloat16)
            nc.sync.dma_start(wdb_t[:], wdb[:])
            idr_t = constp.tile([P, 2, P], dt.float8e4)
            nc.sync.dma_start(idr_t[:], idr[:].rearrange("p (k q) -> p k q", k=2))
            bias_t = constp.tile([P, HC], dt.float32)
            nc.sync.dma_start(bias_t[:], bias_rep[:])
            npad_t = constp.tile([P, NB], dt.bfloat16)
            nc.sync.dma_start(npad_t[:], npad[:])
            ident = constp.tile([P, P], dt.bfloat16)
            make_identity(nc, ident[:])

            aggsb = residp.tile([P, NB * TD2], dt.float32)
            obuf = residp.tile([P, NB * HC], dt.float32)
            obuf16 = residp.tile([P, NB * HC], dt.float16)
            smbuf = residp.tile([P, NB], dt.float32)
            lnb_t = residp.tile([P, NB], dt.float32)

            with (
                tc.tile_pool(name="p2x", bufs=BX) as p2x,
                tc.tile_pool(name="p2", bufs=B2) as p2,
                tc.tile_pool(name="p2f", bufs=2) as p2f,
                tc.tile_pool(name="asrcps", bufs=2, space="PSUM") as asrcp,
                tc.tile_pool(name="heps", bufs=BH, space="PSUM") as hepsp,
                tc.tile_pool(name="aggps", bufs=2, space="PSUM") as aggpsp,
            ):
                # prefetch the first KPRE edge blocks
                pre = []
                xoff = 0
                for jb in range(KPRE):
                    L = int(L_sched[jb])
                    t = p2xpre.tile([P, 2 * L * P], dt.float8e4, tag="xpre")
                    nc.sync.dma_start(
                        t[:], xeT[xoff:xoff + P * 2 * L * P].rearrange("(a b) -> a b", b=2 * L * P))
                    xoff += P * 2 * L * P
                    pre.append(t)

                for jb in range(NB):
                    L = int(L_sched[jb])
                    if jb < KPRE:
                        xta = pre[jb]
                    else:
                        xta = p2x.tile([P, 2 * L * P], dt.float8e4, tag="xta")
                        nc.sync.dma_start(
                            xta[:], xeT[xoff:xoff + P * 2 * L * P].rearrange("(a b) -> a b", b=2 * L * P))
                        xoff += P * 2 * L * P
                    xv = xta[:].rearrange("p (k l q) -> p k l q", k=2, q=P)

                    # z = a_src + a_dst directly in one PSUM bank: the l=0
                    # (self-loop) slice carries x_own, so it pre-fills a_dst
                    # via a broadcast-tiled Wtd rhs; DoubleRow a_src follows
                    asrc = asrcp.tile([P, L * H], dt.float32, space="PSUM", tag="asrc")
                    nc.tensor.matmul(asrc[:], lhsT=xv[:, 0, 0, :],
                                     rhs=wdb_t[:, 0, :].unsqueeze(1).to_broadcast([P, L, H]),
                                     start=True, stop=False, skip_group_check=True)
                    nc.tensor.matmul(asrc[:], lhsT=xv[:, 1, 0, :],
                                     rhs=wdb_t[:, 1, :].unsqueeze(1).to_broadcast([P, L, H]),
                                     start=False, stop=False, skip_group_check=True)
                    for l in range(L):
                        nc.tensor.matmul(asrc[:, l * H:(l + 1) * H],
                                         lhsT=xv[:, :, l, :], rhs=was8_t[:],
                                         perf_mode=DR,
                                         start=False, stop=(l == L - 1), skip_group_check=True)
                    # e = exp(leaky(z)/4) straight from PSUM (Prelu and Exp share a table)
                    lr = p2.tile([P, L, H], dt.float32, tag="lr")
                    nc.scalar.activation(lr[:], asrc[:].rearrange("p (l h) -> p l h", h=H),
                                         AF.Prelu, alpha=NEG_SLOPE)
                    eb = p2.tile([P, L, H], dt.bfloat16, tag="eb")
                    nc.scalar.activation(eb[:], lr[:], AF.Exp, scale=ISC)

                    agg = aggpsp.tile([P, TD2], dt.float32, space="PSUM", tag="agg")
                    # per-node a_dst into spare agg columns (for the pad fix)
                    nc.tensor.matmul(agg[:, TD:TD2], lhsT=xv[:, :, 0, :], rhs=wtd8_t[:],
                                     perf_mode=DR, start=True, stop=False, skip_group_check=True)
                    # e-part aggregation: agg[:, 64:72] += I.T @ e_l
                    for l in range(L):
                        nc.tensor.matmul(agg[:, HC:TD], lhsT=ident[:], rhs=eb[:, l, :],
                                         start=False, stop=False, skip_group_check=True)
                    # h-part: per 8-edge group, DoubleRow h matmuls -> weight -> aggregate
                    for ch0 in range(0, L, GP8):
                        gl = min(GP8, L - ch0)
                        ps = hepsp.tile([P, gl * HC], dt.float32, space="PSUM", tag="heps")
                        for li in range(gl):
                            l = ch0 + li
                            nc.tensor.matmul(ps[:, li * HC:(li + 1) * HC],
                                             lhsT=xv[:, :, l, :], rhs=wf8_t[:],
                                             perf_mode=DR,
                                             start=(li == 0), stop=(li == gl - 1),
                                             skip_group_check=True)
                        w = p2.tile([P, gl, HC], dt.float8e4, tag="w")
                        nc.vector.tensor_tensor(
                            out=w[:].rearrange("p l (h c) -> p l h c", c=C),
                            in0=ps[:].rearrange("p (l h c) -> p l h c", h=H, c=C),
                            in1=eb[:, ch0:ch0 + gl, :].unsqueeze(3).to_broadcast([P, gl, H, C]),
                            op=OP.mult)
                        ng2 = gl // 2
                        for i in range(ng2):
                            l = ch0 + 2 * i
                            nc.tensor.matmul(agg[:, 0:HC], lhsT=idr_t[:],
                                             rhs=w[:, 2 * i:2 * i + 2, :], perf_mode=DR,
                                             start=False, stop=(l + 2 >= L), skip_group_check=True)
                        if gl % 2:
                            nc.tensor.matmul(agg[:, 0:HC], lhsT=ident[:], rhs=w[:, gl - 1, :],
                                             start=False, stop=(ch0 + gl >= L), skip_group_check=True)
                    # park [m | s | a_dst] in SBUF; normalization runs in block groups
                    nc.scalar.copy(out=aggsb[:, jb * TD2:(jb + 1) * TD2], in_=agg[:])

                    # ---------------- grouped finalize (overlaps the loop) ----------------
                    # deferred 2 blocks so its cross-engine chain lands with
                    # dependencies already satisfied (engine queues are in-order)
                    fg = jb - 2
                    if fg >= 0 and (fg + 1) % GB == 0:
                        jb_f = fg
                        g0 = fg + 1 - GB
                        av = aggsb[:, g0 * TD2:(jb_f + 1) * TD2].rearrange("p (t d) -> p t d", d=TD2)
                        # pad correction from the parked per-node a_dst
                        lrg = p2f.tile([P, GB, H], dt.float32, tag="lrg")
                        nc.scalar.activation(lrg[:], av[:, :, TD:TD2], AF.Prelu, alpha=NEG_SLOPE)
                        edg = p2f.tile([P, GB, H], dt.float32, tag="edg")
                        nc.scalar.activation(edg[:], lrg[:], AF.Exp, scale=ISC)
                        pcor = p2f.tile([P, GB, H], dt.float32, tag="pcor")
                        nc.vector.tensor_tensor(
                            out=pcor[:], in0=edg[:],
                            in1=npad_t[:, g0:jb_f + 1].unsqueeze(2).to_broadcast([P, GB, H]),
                            op=OP.mult)
                        nc.vector.tensor_scalar(out=pcor[:], in0=pcor[:],
                                                scalar1=1e-16, scalar2=None, op0=OP.subtract)
                        sden = p2f.tile([P, GB, H], dt.float32, tag="sden")
                        nc.gpsimd.tensor_tensor(out=sden[:], in0=av[:, :, HC:TD],
                                                in1=pcor[:], op=OP.subtract)
                        srec = p2f.tile([P, GB, H], dt.float32, tag="srec")
                        nc.vector.reciprocal(srec[:], sden[:])
                        # undo the x4 h-weight scale here: onorm = m / (4 s)
                        srek = p2f.tile([P, GB, H], dt.float32, tag="srek")
                        nc.vector.tensor_scalar(out=srek[:], in0=srec[:],
                                                scalar1=1.0 / SC_H, scalar2=None, op0=OP.mult)
                        ov = obuf[:, g0 * HC:(jb_f + 1) * HC].rearrange("p (t d) -> p t d", d=HC)
                        nc.gpsimd.tensor_tensor(
                            out=ov.rearrange("p t (h c) -> p t h c", c=C),
                            in0=av[:, :, 0:HC].rearrange("p t (h c) -> p t h c", c=C),
                            in1=srek[:].unsqueeze(3).to_broadcast([P, GB, H, C]),
                            op=OP.mult)
                        nc.gpsimd.tensor_tensor(
                            out=ov, in0=ov,
                            in1=bias_t[:].unsqueeze(1).to_broadcast([P, GB, HC]), op=OP.add)
                        exf = p2f.tile([P, GB, HC], dt.bfloat16, tag="exf")
                        nc.scalar.activation(exf[:], ov, AF.Exp)
                        nc.vector.tensor_reduce(
                            smbuf[:, g0:jb_f + 1].unsqueeze(2), exf[:],
                            axis=mybir.AxisListType.X, op=OP.add)

                    # early log-softmax flush: everything finalized so far
                    # ships while the last blocks still compute
                    if jb == NCUT + 1:
                        nc.scalar.activation(lnb_t[:, 0:NCUT], smbuf[:, 0:NCUT], AF.Ln)
                        with nc.allow_low_precision(reason="fp16 output"):
                            nc.vector.tensor_tensor(
                                out=obuf16[:, 0:NCUT * HC].rearrange("p (t d) -> p t d", d=HC),
                                in0=obuf[:, 0:NCUT * HC].rearrange("p (t d) -> p t d", d=HC),
                                in1=lnb_t[:, 0:NCUT].unsqueeze(2).to_broadcast([P, NCUT, HC]),
                                op=OP.subtract)
                        nc.sync.dma_start(out[:, 0:NCUT * HC], obuf16[:, 0:NCUT * HC])

                # last group's finalize (its 2-block-deferred trigger is past the loop)
                jb_f = NB - 1
                g0 = NB - GB
                av = aggsb[:, g0 * TD2:NB * TD2].rearrange("p (t d) -> p t d", d=TD2)
                lrg = p2f.tile([P, GB, H], dt.float32, tag="lrg")
                nc.scalar.activation(lrg[:], av[:, :, TD:TD2], AF.Prelu, alpha=NEG_SLOPE)
                edg = p2f.tile([P, GB, H], dt.float32, tag="edg")
                nc.scalar.activation(edg[:], lrg[:], AF.Exp, scale=ISC)
                pcor = p2f.tile([P, GB, H], dt.float32, tag="pcor")
                nc.vector.tensor_tensor(
                    out=pcor[:], in0=edg[:],
                    in1=npad_t[:, g0:NB].unsqueeze(2).to_broadcast([P, GB, H]),
                    op=OP.mult)
                nc.vector.tensor_scalar(out=pcor[:], in0=pcor[:],
                                        scalar1=1e-16, scalar2=None, op0=OP.subtract)
                sden = p2f.tile([P, GB, H], dt.float32, tag="sden")
                nc.gpsimd.tensor_tensor(out=sden[:], in0=av[:, :, HC:TD],
                                        in1=pcor[:], op=OP.subtract)
                srec = p2f.tile([P, GB, H], dt.float32, tag="srec")
                nc.vector.reciprocal(srec[:], sden[:])
                srek = p2f.tile([P, GB, H], dt.float32, tag="srek")
                nc.vector.tensor_scalar(out=srek[:], in0=srec[:],
                                        scalar1=1.0 / SC_H, scalar2=None, op0=OP.mult)
                ov = obuf[:, g0 * HC:NB * HC].rearrange("p (t d) -> p t d", d=HC)
                nc.gpsimd.tensor_tensor(
                    out=ov.rearrange("p t (h c) -> p t h c", c=C),
                    in0=av[:, :, 0:HC].rearrange("p t (h c) -> p t h c", c=C),
                    in1=srek[:].unsqueeze(3).to_broadcast([P, GB, H, C]),
                    op=OP.mult)
                nc.gpsimd.tensor_tensor(
                    out=ov, in0=ov,
                    in1=bias_t[:].unsqueeze(1).to_broadcast([P, GB, HC]), op=OP.add)
                exf = p2f.tile([P, GB, HC], dt.bfloat16, tag="exf")
                nc.scalar.activation(exf[:], ov, AF.Exp)
                nc.vector.tensor_reduce(
                    smbuf[:, g0:NB].unsqueeze(2), exf[:],
                    axis=mybir.AxisListType.X, op=OP.add)

                # ---------------- deferred log-softmax tail (last groups) ----------------
                NREM = NB - NCUT
                nc.scalar.activation(lnb_t[:, NCUT:NB], smbuf[:, NCUT:NB], AF.Ln)
                with nc.allow_low_precision(reason="fp16 output"):
                    nc.vector.tensor_tensor(
                        out=obuf16[:, NCUT * HC:].rearrange("p (t d) -> p t d", d=HC),
                        in0=obuf[:, NCUT * HC:].rearrange("p (t d) -> p t d", d=HC),
                        in1=lnb_t[:, NCUT:NB].unsqueeze(2).to_broadcast([P, NREM, HC]),
                        op=OP.subtract)
                nc.sync.dma_start(out[:, NCUT * HC:], obuf16[:, NCUT * HC:])

    nc.compile()
    return nc


def kernel(x, edge_index, W, att_src, att_dst, bias):
    in_maps, L_sched, S, row2node = _host_prep(x, edge_index, W, att_src, att_dst, bias)
    nc = _build_program(L_sched, S)
    res = run_bass_kernel_spmd(nc, in_maps, core_ids=list(range(NCORES)))
    out_full = np.empty((N, HC), dtype=np.float32)
    for cc in range(NCORES):
        o = np.asarray(res.results[cc]["out"]).astype(np.float32)   # [128, NB*HC]
        o = o.reshape(P, NB, HC).transpose(1, 0, 2).reshape(NPC, HC)
        rr = row2node[cc * NPC:(cc + 1) * NPC]
        m = rr >= 0
        out_full[rr[m]] = o[m]
    return out_full
